# revision 1
# baseline (speedup 1.0000x reference)
"""CapsuleLayer (dynamic routing) Trainium2 kernel, 8-core SPMD.

Problem (hardcoded):
  x: [B=64, I=2048, D=128] f32, W: [I=2048, J=32, D=128, V=32] f32
  u_hat[b,i,j,v] = sum_d W[i,j,d,v] x[b,i,d]
  3 routing iterations; b_ij updated with mean-over-batch agreement,
  so c_ij is identical across batch -> c is effectively [I, J].
  output v_j: [B, J=32, V=32] f32.

Sharding: capsule-input dim I split across 8 cores (256 each). Per
routing iteration the only cross-core quantities are the [B,J,V]
s-partials and the [J] softmax denominators -> fused into one
AllReduce per iteration (3 total).

Device layout: the (j,v) axes of u_hat are stored "split" so every
tile uses all 128 partitions:
  partition p = vh*64 + b   (vh = v // 16, b = batch)
  free     f  = vl*32 + j   (vl = v % 16)
u_hat is cached in DRAM as fp16.
"""

import numpy as np

B = 64
I = 2048
J = 32
D = 128
V = 32
NC = 8
IL = I // NC          # capsules-in per core
G = 64                # i-block size for routing passes
JV = J * V            # 1024
F = JV // 2           # 512 free size in split layout
VL = 16               # v-low values per partition half
EPS = 1e-9
EXP_SHIFT = 0.0       # softmax shift (global constant, cancels in softmax)
ALPHA_A = 1.0 / 16.0  # pass-A fp16 e-scale (cancels via 1/(alpha*Z))

_CACHE = {}


def _apply_tile_patch(tile_mod, mybir):
    """walrus in this toolchain rejects >1 sem-wait per instruction; split
    extra waits onto same-engine NoOps placed just before the instruction,
    and split the TileContext tail-drain waits across sequential drains."""
    import bass_rust
    from concourse.vector_clock import ScopedClock

    if getattr(tile_mod.TileContext, "_wait_split_patched", False):
        return

    _orig_lower = tile_mod.TileContext._lower_ordered_insts
    _ctr = [0]

    def _split_waits(self, ordered):
        for bb_name, insts in ordered.items():
            new_insts = []
            for inst in insts:
                si = getattr(inst, "sync_info", None)
                waits = list(si.on_wait) if si is not None and si.on_wait else []
                eng = getattr(inst, "engine", None)
                if len(waits) > 1 and eng is not None:
                    for w in waits[:-1]:
                        _ctr[0] += 1
                        nop = bass_rust.InstNoOp(
                            name=f"I-wsplit-{_ctr[0]}", ins=[], outs=[]
                        )
                        nop.engine = eng
                        nop.sync_info = mybir.SyncInfo(on_wait=[w], on_update=[])
                        new_insts.append(nop)
                    si.on_wait = waits[-1:]
                new_insts.append(inst)
            insts[:] = new_insts
        return _orig_lower(self, ordered)

    tile_mod.TileContext._lower_ordered_insts = _split_waits
    tile_mod.TileContext._wait_split_patched = True

    def _patched(self, tick_clock, wait_clock):
        nc = self.nc
        drain_inst = nc.sync.drain()
        wait_clock.add_sem_waits(
            drain_inst.ins, ScopedClock({None: tick_clock.global_clock})
        )
        si = drain_inst.ins.sync_info
        if si is not None and si.on_wait and len(si.on_wait) > 1:
            waits = list(si.on_wait)
            si.on_wait = waits[:1]
            for k in range(1, len(waits)):
                extra = nc.sync.drain()
                extra.ins.sync_info = mybir.SyncInfo(
                    on_wait=waits[k : k + 1], on_update=[]
                )
        nc.all_engine_barrier()
        assert self.sems is not None
        popped = nc._tile_sem_poison_stack.pop()
        assert popped is self._sem_poison
        nc.clear_and_free_semaphores(list(self.sems.allocated().values()))
        nc.all_engine_barrier()

    tile_mod.TileContext._drain_and_barrier = _patched


def build_kernel(il=IL, g=G, n_cores=NC, debug=False, passa_bf16=False, fake_cc=False):
    from concourse import bass, mybir, tile

    _apply_tile_patch(tile, mybir)

    nblk = il // g
    DT16 = mybir.dt.float16
    DTB = mybir.dt.bfloat16
    DT32 = mybir.dt.float32
    AF = mybir.ActivationFunctionType
    OP = mybir.AluOpType
    AX = mybir.AxisListType
    cores = list(range(n_cores))

    nc = bass.Bass()
    xt = nc.declare_dram_parameter("xt", [il, D, B], DT16, isOutput=False)
    wt = nc.declare_dram_parameter("wt", [il, D, JV], DT16, isOutput=False)
    out = nc.declare_dram_parameter("out", [2, B, VL, J], DT32, isOutput=True)
    u_dram = nc.dram_tensor("u_dram", [il, 128, F], DT16)
    if debug:
        dbg_v0 = nc.declare_dram_parameter("dbg_v0", [128, F], DT16, isOutput=True)
        dbg_bsb = nc.declare_dram_parameter("dbg_bsb", [1, J * g], DT32, isOutput=True)
        dbg_esb = nc.declare_dram_parameter("dbg_esb", [1, J * g], DT32, isOutput=True)
        dbg_eb16 = nc.declare_dram_parameter("dbg_eb16", [128, g * J], DT16, isOutput=True)
        dbg_s0 = nc.declare_dram_parameter("dbg_s0", [128, F], DT32, isOutput=True)
        dbg_b2 = nc.declare_dram_parameter("dbg_b2", [1, J * g], DT32, isOutput=True)
        dbg_b1s = nc.declare_dram_parameter("dbg_b1s", [1, J * g], DT32, isOutput=True)
        dbg_v1 = nc.declare_dram_parameter("dbg_v1", [128, F], DT16, isOutput=True)
        dbg_agp = nc.declare_dram_parameter("dbg_agp", [1, J * g], DT32, isOutput=True)

    with tile.TileContext(nc) as tc:
        with (
            tc.tile_pool(name="keep", bufs=1) as keep,
            tc.tile_pool(name="cdram", bufs=1, space="DRAM") as cdram,
        ):
            ones_row = keep.tile([1, 128], DT32, tag="ones_row")
            nc.gpsimd.memset(ones_row[:], 1.0)
            epsb = keep.tile([128, 1], DT32, tag="epsb")
            nc.gpsimd.memset(epsb[:], EPS)
            shiftb = keep.tile([1, 1], DT32, tag="shiftb")
            nc.gpsimd.memset(shiftb[:], -EXP_SHIFT)

            def squash(s32, scale, out_dtype):
                """v = squash(s32*scale) in split layout; reduces over j."""
                sq = keep.tile([128, F], DT32, tag="sq_tmp")
                nc.scalar.activation(sq[:], s32[:], AF.Square, scale=float(scale))
                magsq = keep.tile([128, VL], DT32, tag="sq_magsq")
                nc.vector.tensor_reduce(
                    magsq[:], sq[:].rearrange("p (vl j) -> p vl j", j=J), AX.X, OP.add
                )
                mag = keep.tile([128, VL], DT32, tag="sq_mag")
                nc.scalar.activation(mag[:], magsq[:], AF.Sqrt, bias=epsb[:])
                onep = keep.tile([128, VL], DT32, tag="sq_onep")
                nc.vector.tensor_scalar_add(onep[:], magsq[:], 1.0)
                den = keep.tile([128, VL], DT32, tag="sq_den")
                nc.vector.tensor_tensor(den[:], onep[:], mag[:], OP.mult)
                rden = keep.tile([128, VL], DT32, tag="sq_rden")
                nc.vector.reciprocal(rden[:], den[:])
                fq = keep.tile([128, VL], DT32, tag="sq_f")
                nc.vector.tensor_tensor(fq[:], magsq[:], rden[:], OP.mult)
                fs = keep.tile([128, VL], DT32, tag="sq_fs")
                nc.vector.tensor_scalar_mul(fs[:], fq[:], float(scale))
                v = keep.tile([128, F], out_dtype, tag="sq_v" + str(out_dtype))
                nc.vector.tensor_tensor(
                    v[:].rearrange("p (vl j) -> p vl j", j=J),
                    s32[:].rearrange("p (vl j) -> p vl j", j=J),
                    fs[:].broadcast_to([128, VL, J]),
                    OP.mult,
                )
                return v

            def allreduce_s(k, s_sb, e_sb=None):
                # pack [128,F] s-partial and the [J] softmax denominator
                # into one flat buffer -> a single AllReduce per iteration
                ntot = 128 * F + J
                s_in = cdram.tile([ntot], DT32, tag=f"sin{k}")
                s_out = cdram.tile([ntot], DT32, tag=f"sout{k}")
                nc.scalar.dma_start(
                    s_in[0 : 128 * F].rearrange("(p f) -> p f", f=F), s_sb[:]
                )
                if e_sb is not None:
                    nc.scalar.dma_start(
                        s_in[128 * F : ntot].rearrange("(o j) -> o j", j=J),
                        e_sb[:],
                    )
                else:
                    # keep the unused tail deterministic for the reduce
                    ztail = keep.tile([1, J], DT32, tag="ztail")
                    nc.gpsimd.memset(ztail[:], 0.0)
                    nc.scalar.dma_start(
                        s_in[128 * F : ntot].rearrange("(o j) -> o j", j=J),
                        ztail[:],
                    )
                if fake_cc:
                    nc.gpsimd.dma_start(s_out[:], s_in[:])
                else:
                    nc.gpsimd.collective_compute(
                        "AllReduce",
                        OP.add,
                        replica_groups=[cores],
                        ins=[s_in.opt()],
                        outs=[s_out.opt()],
                    )
                sg = keep.tile([128, F], DT32, tag=f"ar_sg{k}")
                nc.scalar.dma_start(
                    sg[:], s_out[0 : 128 * F].rearrange("(p f) -> p f", f=F)
                )
                zg = None
                if e_sb is not None:
                    zg = keep.tile([1, J], DT32, tag=f"ar_zg{k}")
                    nc.scalar.dma_start(
                        zg[:], s_out[128 * F : ntot].rearrange("(o j) -> o j", j=J)
                    )
                return sg, zg

            # ---------------- Phase 1: u_hat matmul + s0 accumulation ----
            with (
                tc.tile_pool(name="xpool", bufs=1) as xpool,
                tc.tile_pool(name="wpool", bufs=4) as wpool,
                tc.tile_pool(name="upool", bufs=4) as upool,
                tc.tile_pool(name="psU", bufs=3, space="PSUM") as psU,
                tc.tile_pool(name="psS", bufs=1, space="PSUM") as psS,
            ):
                xs = xpool.tile([D, il, B], DT16)
                nc.sync.dma_start(xs[:], xt[:].rearrange("i d b -> d i b"))
                s0lo = psS.tile([128, F], DT32, tag="s0lo")
                s0hi = psS.tile([128, F], DT32, tag="s0hi")
                for i in range(il):
                    wtile = wpool.tile([D, JV], DT16)
                    nc.sync.dma_start(wtile[:], wt[i])
                    ups = psU.tile([128, F], DT32)
                    lhs = xs[:, i, :]
                    nc.tensor.matmul(
                        ups[0:64, :], lhs, wtile[:, 0:F], start=True, stop=True
                    )
                    nc.tensor.matmul(
                        ups[64:128, :],
                        lhs,
                        wtile[:, F:JV],
                        start=True,
                        stop=True,
                        tile_position=(0, 64),
                    )
                    nc.tensor.matmul(
                        s0lo[0:64, :],
                        lhs,
                        wtile[:, 0:F],
                        start=(i == 0),
                        stop=(i == il - 1),
                    )
                    nc.tensor.matmul(
                        s0hi[64:128, :],
                        lhs,
                        wtile[:, F:JV],
                        start=(i == 0),
                        stop=(i == il - 1),
                        tile_position=(0, 64),
                    )
                    ut = upool.tile([128, F], DT16)
                    nc.vector.tensor_copy(ut[:], ups[:])
                    nc.gpsimd.dma_start(u_dram[i], ut[:])
                s0sb = keep.tile([128, F], DT32, tag="s0sb")
                nc.vector.tensor_copy(s0sb[0:64, :], s0lo[0:64, :])
                nc.vector.tensor_copy(s0sb[64:128, :], s0hi[64:128, :])

            s0g, _ = allreduce_s(0, s0sb)
            v0 = squash(s0g, 1.0 / float(il * n_cores), DT16)
            if debug:
                nc.sync.dma_start(dbg_s0[:], s0g[:])
                nc.sync.dma_start(dbg_v0[:], v0[:])

            # ------------- Routing passes (agree -> b -> e -> s-partial) --
            bdram = cdram.tile([nblk, J * g], DT32, tag="bdram")

            def routing_pass(vprev16, first):
                tag = "A" if first else "B"
                szu = keep.tile([128, F], DT32, tag="szu" + tag)
                eloc = keep.tile([1, J], DT32, tag="eloc" + tag)
                with (
                    tc.tile_pool(name="ubpool", bufs=2) as ubpool,
                    tc.tile_pool(name="ebpool", bufs=2) as ebpool,
                    tc.tile_pool(name="prpool", bufs=1) as prpool,
                    tc.tile_pool(name="smallp", bufs=1) as smallp,
                    tc.tile_pool(name="psAg", bufs=1, space="PSUM") as psAg,
                    tc.tile_pool(name="psEb", bufs=1, space="PSUM") as psEb,
                ):
                    for bk in range(nblk):
                        ublk = ubpool.tile([128, g, F], DT16)
                        nc.sync.dma_start(
                            ublk[:],
                            u_dram[bk * g : (bk + 1) * g].rearrange("g p f -> p g f"),
                        )
                        # agreement: b1[i,j]*B = sum_{b,v} u_hat * v_prev
                        # one K=128 matmul per (j, vl), accumulated over vl
                        agps = psAg.tile([1, J * g], DT32)
                        for j in range(J):
                            for vl in range(VL):
                                col = vl * J + j
                                nc.tensor.matmul(
                                    agps[:, j * g : (j + 1) * g],
                                    vprev16[:, col : col + 1],
                                    ublk[:, :, col],
                                    start=(vl == 0),
                                    stop=(vl == VL - 1),
                                )
                        # b_k = agree/B (+ b_prev); e = exp(b_k - SHIFT)
                        bsb = smallp.tile([1, J * g], DT32, tag="bsbB")
                        if first:
                            nc.scalar.mul(bsb[:], agps[:], 1.0 / B)
                            nc.sync.dma_start(
                                bdram[bk].rearrange("(o c) -> o c", o=1), bsb[:]
                            )
                        else:
                            b1blk = smallp.tile([1, J * g], DT32, tag="rowA")
                            nc.sync.dma_start(
                                b1blk[:], bdram[bk].rearrange("(o c) -> o c", o=1)
                            )
                            nc.vector.scalar_tensor_tensor(
                                bsb[:], agps[:], 1.0 / B, b1blk[:],
                                OP.mult, OP.add,
                            )
                        esb = smallp.tile([1, J * g], DT32, tag="rowA")
                        nc.scalar.activation(esb[:], bsb[:], AF.Exp, bias=shiftb[:])
                        # local softmax denominator accumulation
                        etmp = smallp.tile([1, J], DT32, tag="etmp")
                        nc.vector.tensor_reduce(
                            etmp[:],
                            esb[:].rearrange("p (j g) -> p j g", g=g),
                            AX.X,
                            OP.add,
                        )
                        if bk == 0:
                            nc.vector.tensor_copy(eloc[:], etmp[:])
                        else:
                            nc.vector.tensor_tensor(eloc[:], eloc[:], etmp[:], OP.add)
                        # broadcast e to all partitions (i-major) via
                        # K=1 ones-matmuls, then downcast to fp16
                        ebps = psEb.tile([128, g * J], DT32)
                        eview = (
                            esb[:]
                            .rearrange("p (j g) -> p j g", g=g)
                            .rearrange("p j g -> p g j")
                        )
                        gchunk = min(g, 512 // J)
                        for k in range(0, g, gchunk):
                            nc.tensor.matmul(
                                ebps[:, k * J : (k + gchunk) * J],
                                ones_row[:],
                                eview[:, k : k + gchunk, :],
                                start=True,
                                stop=True,
                            )
                        # pass A: e fits fp16 (b1 in [-5,6] at full size),
                        # scaled by 2^-4 for headroom (cancels via 1/Z).
                        # pass B: b2 can reach ~33, so e spans e^44 -> bf16
                        # for e AND the products/tree (range, not precision)
                        edt = (DTB if passa_bf16 else DT16) if first else DTB
                        alpha = ALPHA_A if first else 1.0
                        eb16 = ebpool.tile([128, g, 1, J], edt, tag="eb16")
                        nc.scalar.mul(
                            eb16[:].rearrange("p g o j -> p (g o j)"),
                            ebps[:],
                            alpha,
                        )
                        if debug and first and bk == 0:
                            nc.sync.dma_start(dbg_bsb[:], bsb[:])
                            nc.sync.dma_start(dbg_esb[:], esb[:])
                        # products into a quarter-block tile, tree-sum over i
                        gh = max(1, g // 4)
                        for hh in range(g // gh):
                            prods = prpool.tile([128, gh, F], edt, tag="prods")
                            sl = slice(hh * gh, (hh + 1) * gh)
                            nc.vector.tensor_tensor(
                                prods[:].rearrange("p g (vl j) -> p g vl j", j=J),
                                ublk[:, sl, :].rearrange(
                                    "p g (vl j) -> p g vl j", j=J
                                ),
                                eb16[:, sl, :, :].broadcast_to([128, gh, VL, J]),
                                OP.mult,
                            )
                            h = gh
                            while h > 1:
                                h //= 2
                                nc.vector.tensor_tensor(
                                    prods[:, 0:h, :],
                                    prods[:, 0:h, :],
                                    prods[:, h : 2 * h, :],
                                    OP.add,
                                )
                            p32 = smallp.tile([128, F], DT32, tag="p32")
                            nc.vector.tensor_copy(p32[:], prods[:, 0, :])
                            if bk == 0 and hh == 0:
                                nc.vector.tensor_copy(szu[:], p32[:])
                            else:
                                nc.vector.tensor_tensor(
                                    szu[:], szu[:], p32[:], OP.add
                                )
                return szu, eloc

            def normalize(k, sg, zg, alpha):
                """s = s_unnorm / (alpha*Z) per j."""
                zr = keep.tile([1, J], DT32, tag=f"zr{k}")
                nc.vector.reciprocal(zr[:], zg[:])
                if alpha != 1.0:
                    nc.vector.tensor_scalar_mul(zr[:], zr[:], 1.0 / alpha)
                zb = keep.tile([128, 1, J], DT32, tag=f"zb{k}")
                with tc.tile_pool(name="psZ", bufs=1, space="PSUM") as psZ:
                    zbps = psZ.tile([128, J], DT32, tag=f"zbps{k}")
                    nc.tensor.matmul(
                        zbps[:], ones_row[:], zr[:], start=True, stop=True
                    )
                    nc.vector.tensor_copy(
                        zb[:].rearrange("p o j -> p (o j)"), zbps[:]
                    )
                s = keep.tile([128, F], DT32, tag=f"snorm{k}")
                nc.vector.tensor_tensor(
                    s[:].rearrange("p (vl j) -> p vl j", j=J),
                    sg[:].rearrange("p (vl j) -> p vl j", j=J),
                    zb[:].broadcast_to([128, VL, J]),
                    OP.mult,
                )
                return s

            s1u, e1 = routing_pass(v0, True)
            s1g, z1 = allreduce_s(1, s1u, e1)
            v1 = squash(normalize(1, s1g, z1, ALPHA_A), 1.0, DT16)

            s2u, e2 = routing_pass(v1, False)
            s2g, z2 = allreduce_s(2, s2u, e2)
            v2 = squash(normalize(2, s2g, z2, 1.0), 1.0, DT32)

            # output in device layout [vh, b, vl, j]; host transposes
            for vh in range(2):
                nc.sync.dma_start(
                    out[vh].rearrange("b vl j -> b (vl j)"),
                    v2[vh * 64 : (vh + 1) * 64, :],
                )
    return nc


def _prep_inputs(x, W, n_cores=NC):
    il = x.shape[1] // n_cores
    x16 = x.astype(np.float16)
    W16 = W.astype(np.float16)
    maps = []
    for c in range(n_cores):
        sl = slice(c * il, (c + 1) * il)
        xtc = np.ascontiguousarray(x16[:, sl, :].transpose(1, 2, 0))  # [il,D,B]
        wtc = np.ascontiguousarray(W16[sl].transpose(0, 2, 3, 1)).reshape(
            il, D, JV
        )  # [il, D, (v,j)]
        maps.append({"xt": xtc, "wt": wtc})
    return maps


def kernel(x, W):
    from concourse.bass_utils import run_bass_kernel_spmd

    if "nc" not in _CACHE:
        _CACHE["nc"] = build_kernel()
    in_maps = _prep_inputs(np.asarray(x), np.asarray(W))
    res = run_bass_kernel_spmd(_CACHE["nc"], in_maps, list(range(NC)))
    o = np.asarray(res.results[0]["out"], dtype=np.float32)  # [2, B, VL, J]
    return np.ascontiguousarray(o.transpose(1, 3, 0, 2).reshape(B, J, V))



# revision 20
# speedup vs baseline: 62.5487x; 62.5487x over previous
"""CapsuleLayer (dynamic routing) Trainium2 kernel, 8-core SPMD. v2.

Problem (hardcoded):
  x: [B=64, I=2048, D=128] f32, W: [I=2048, J=32, D=128, V=32] f32
  u_hat[b,i,j,v] = sum_d W[i,j,d,v] x[b,i,d]
  3 routing iterations; b_ij updated with mean-over-batch agreement,
  so c_ij is identical across batch -> c is effectively [I, J].
  output v_j: [B, J=32, V=32] f32.

Sharding: capsule-input dim I split across 8 cores (256 each). Per
routing iteration the only cross-core quantities are the [B,J,V]
s-partials and the [J] softmax denominators -> fused into one
AllReduce per iteration (3 total).

Device layout: the (j,v) axes of u_hat are stored "split" so every
tile uses all 128 partitions:
  partition p = vh*64 + b   (vh = v // 16, b = batch)
  free     f  = vl*32 + j   (vl = v % 16)

v2 changes vs baseline:
  - u_hat cached in DRAM partition-major [128, il, F] so routing-pass
    loads are one contiguous 64KB-per-partition DMA per block.
  - x staged host-side as [D, il, B] (device DMA fully contiguous).
  - W loads batched 4 capsules per DMA (1 MB transfers).
  - PSUM evacuation split between ScalarE and VectorE; u-store DMAs
    moved to the ACT HWDGE ring, 4 capsules per DMA.
"""

import numpy as np

B = 64
I = 2048
J = 32
D = 128
V = 32
NC = 8
IL = I // NC          # capsules-in per core
G = 64                # i-block size for routing passes
JV = J * V            # 1024
F = JV // 2           # 512 free size in split layout
VL = 16               # v-low values per partition half
EPS = 1e-9
EXP_SHIFT = 0.0       # softmax shift (global constant, cancels in softmax)
ALPHA_A = 1.0 / 16.0  # pass-A fp16 e-scale (cancels via 1/(alpha*Z))

_CACHE = {}


def _apply_tile_patch(tile_mod, mybir):
    """walrus in this toolchain rejects >1 sem-wait per instruction; split
    extra waits onto same-engine NoOps placed just before the instruction,
    and split the TileContext tail-drain waits across sequential drains."""
    import bass_rust
    from concourse.vector_clock import ScopedClock

    if getattr(tile_mod.TileContext, "_wait_split_patched", False):
        return

    _orig_lower = tile_mod.TileContext._lower_ordered_insts
    _ctr = [0]

    def _split_waits(self, ordered):
        for bb_name, insts in ordered.items():
            new_insts = []
            for inst in insts:
                si = getattr(inst, "sync_info", None)
                waits = list(si.on_wait) if si is not None and si.on_wait else []
                eng = getattr(inst, "engine", None)
                if len(waits) > 1 and eng is not None:
                    for w in waits[:-1]:
                        _ctr[0] += 1
                        nop = bass_rust.InstNoOp(
                            name=f"I-wsplit-{_ctr[0]}", ins=[], outs=[]
                        )
                        nop.engine = eng
                        nop.sync_info = mybir.SyncInfo(on_wait=[w], on_update=[])
                        new_insts.append(nop)
                    si.on_wait = waits[-1:]
                new_insts.append(inst)
            insts[:] = new_insts
        return _orig_lower(self, ordered)

    tile_mod.TileContext._lower_ordered_insts = _split_waits
    tile_mod.TileContext._wait_split_patched = True

    def _patched(self, tick_clock, wait_clock):
        nc = self.nc
        drain_inst = nc.sync.drain()
        wait_clock.add_sem_waits(
            drain_inst.ins, ScopedClock({None: tick_clock.global_clock})
        )
        si = drain_inst.ins.sync_info
        if si is not None and si.on_wait and len(si.on_wait) > 1:
            waits = list(si.on_wait)
            si.on_wait = waits[:1]
            for k in range(1, len(waits)):
                extra = nc.sync.drain()
                extra.ins.sync_info = mybir.SyncInfo(
                    on_wait=waits[k : k + 1], on_update=[]
                )
        nc.all_engine_barrier()
        assert self.sems is not None
        popped = nc._tile_sem_poison_stack.pop()
        assert popped is self._sem_poison
        nc.clear_and_free_semaphores(list(self.sems.allocated().values()))
        nc.all_engine_barrier()

    tile_mod.TileContext._drain_and_barrier = _patched


def build_kernel(il=IL, g=G, n_cores=NC, debug=False, passa_bf16=False, fake_cc=False):
    from concourse import bass, mybir, tile

    _apply_tile_patch(tile, mybir)

    nblk = il // g
    DT16 = mybir.dt.float16
    DTB = mybir.dt.bfloat16
    DT32 = mybir.dt.float32
    AF = mybir.ActivationFunctionType
    OP = mybir.AluOpType
    AX = mybir.AxisListType
    cores = list(range(n_cores))

    nc = bass.Bass()
    xt = nc.declare_dram_parameter("xt", [D, il, B], DT16, isOutput=False)
    wt = nc.declare_dram_parameter("wt", [il, D, JV], DT16, isOutput=False)
    out = nc.declare_dram_parameter("out", [2, B, VL, J], DT32, isOutput=True)
    # partition-major u cache: contiguous (i, f) per partition row
    u_dram = nc.dram_tensor("u_dram", [128, il, F], DT16)

    with tile.TileContext(nc) as tc:
        with (
            tc.tile_pool(name="keep", bufs=1) as keep,
            tc.tile_pool(name="cdram", bufs=1, space="DRAM") as cdram,
        ):
            ones_row = keep.tile([1, 128], DT32, tag="ones_row")
            nc.gpsimd.memset(ones_row[:], 1.0)
            epsb = keep.tile([128, 1], DT32, tag="epsb")
            nc.gpsimd.memset(epsb[:], EPS)
            shiftb = keep.tile([1, 1], DT32, tag="shiftb")
            nc.gpsimd.memset(shiftb[:], -EXP_SHIFT)

            def squash(s32, scale, out_dtype):
                """v = squash(s32*scale) in split layout; reduces over j."""
                sq = keep.tile([128, F], DT32, tag="sq_tmp")
                nc.scalar.activation(sq[:], s32[:], AF.Square, scale=float(scale))
                magsq = keep.tile([128, VL], DT32, tag="sq_magsq")
                nc.vector.tensor_reduce(
                    magsq[:], sq[:].rearrange("p (vl j) -> p vl j", j=J), AX.X, OP.add
                )
                mag = keep.tile([128, VL], DT32, tag="sq_mag")
                nc.scalar.activation(mag[:], magsq[:], AF.Sqrt, bias=epsb[:])
                den = keep.tile([128, VL], DT32, tag="sq_den")
                nc.vector.scalar_tensor_tensor(
                    den[:], magsq[:], 1.0, mag[:], OP.add, OP.mult
                )
                rden = keep.tile([128, VL], DT32, tag="sq_rden")
                nc.vector.reciprocal(rden[:], den[:])
                fs = keep.tile([128, VL], DT32, tag="sq_fs")
                nc.vector.scalar_tensor_tensor(
                    fs[:], magsq[:], float(scale), rden[:], OP.mult, OP.mult
                )
                v = keep.tile([128, F], out_dtype, tag="sq_v" + str(out_dtype))
                nc.vector.tensor_tensor(
                    v[:].rearrange("p (vl j) -> p vl j", j=J),
                    s32[:].rearrange("p (vl j) -> p vl j", j=J),
                    fs[:].broadcast_to([128, VL, J]),
                    OP.mult,
                )
                return v

            def allreduce_s(k, s_sb, e_sb=None):
                # pack [128,F] s-partial and the [J] softmax denominator
                # into one flat buffer -> a single AllReduce per iteration
                ntot = 128 * F + J
                s_in = cdram.tile([ntot], DT32, tag=f"sin{k}")
                s_out = cdram.tile([ntot], DT32, tag=f"sout{k}")
                nc.scalar.dma_start(
                    s_in[0 : 128 * F].rearrange("(p f) -> p f", f=F), s_sb[:]
                )
                if e_sb is not None:
                    nc.scalar.dma_start(
                        s_in[128 * F : ntot].rearrange("(o j) -> o j", j=J),
                        e_sb[:],
                    )
                else:
                    # keep the unused tail deterministic for the reduce
                    ztail = keep.tile([1, J], DT32, tag="ztail")
                    nc.gpsimd.memset(ztail[:], 0.0)
                    nc.scalar.dma_start(
                        s_in[128 * F : ntot].rearrange("(o j) -> o j", j=J),
                        ztail[:],
                    )
                if fake_cc:
                    nc.gpsimd.dma_start(s_out[:], s_in[:])
                else:
                    nc.gpsimd.collective_compute(
                        "AllReduce",
                        OP.add,
                        replica_groups=[cores],
                        ins=[s_in.opt()],
                        outs=[s_out.opt()],
                    )
                sg = keep.tile([128, F], DT32, tag=f"ar_sg{k}")
                nc.scalar.dma_start(
                    sg[:], s_out[0 : 128 * F].rearrange("(p f) -> p f", f=F)
                )
                zg = None
                if e_sb is not None:
                    zg = keep.tile([1, J], DT32, tag=f"ar_zg{k}")
                    nc.scalar.dma_start(
                        zg[:], s_out[128 * F : ntot].rearrange("(o j) -> o j", j=J)
                    )
                return sg, zg

            # ---------------- Phase 1: u_hat matmul + s0 accumulation ----
            with (
                tc.tile_pool(name="xpool", bufs=1) as xpool,
                tc.tile_pool(name="wpool", bufs=3) as wpool,
                tc.tile_pool(name="upool", bufs=3) as upool,
                tc.tile_pool(name="psU", bufs=2, space="PSUM") as psU,
                tc.tile_pool(name="psS", bufs=1, space="PSUM") as psS,
            ):
                xs = xpool.tile([D, il, B], DT16)
                nc.sync.dma_start(xs[:], xt[:])
                s0lo = psS.tile([128, F], DT32, tag="s0lo")
                s0hi = psS.tile([128, F], DT32, tag="s0hi")
                for q in range(il // 8):
                    wtile = wpool.tile([D, 8, JV], DT16)
                    nc.sync.dma_start(
                        wtile[:],
                        wt[8 * q : 8 * q + 8].rearrange("i d f -> d i f"),
                    )
                    for h in range(2):
                        up4 = upool.tile([128, 4, F], DT16)
                        for t in range(2):
                            ups = psU.tile([128, 2, F], DT32)
                            for k in range(2):
                                ii = 4 * h + 2 * t + k
                                i = 8 * q + ii
                                lhs = xs[:, i, :]
                                wv = wtile[:, ii, :]
                                nc.tensor.matmul(
                                    ups[0:64, k, :], lhs, wv[:, 0:F],
                                    start=True, stop=True,
                                )
                                nc.tensor.matmul(
                                    ups[64:128, k, :], lhs, wv[:, F:JV],
                                    start=True, stop=True, tile_position=(0, 64),
                                )
                                nc.tensor.matmul(
                                    s0lo[0:64, :], lhs, wv[:, 0:F],
                                    start=(i == 0), stop=(i == il - 1),
                                )
                                nc.tensor.matmul(
                                    s0hi[64:128, :], lhs, wv[:, F:JV],
                                    start=(i == 0), stop=(i == il - 1),
                                    tile_position=(0, 64),
                                )
                            # evacuate this PSUM pair; alternate ACT / DVE
                            dst = up4[:, 2 * t : 2 * t + 2, :].rearrange(
                                "p i f -> p (i f)"
                            )
                            src = ups[:].rearrange("p i f -> p (i f)")
                            if t == 0:
                                nc.scalar.activation(dst, src, AF.Copy)
                            else:
                                nc.vector.tensor_copy(dst, src)
                        nc.scalar.dma_start(
                            u_dram[:, 8 * q + 4 * h : 8 * q + 4 * h + 4, :],
                            up4[:],
                        )
                s0sb = keep.tile([128, F], DT32, tag="s0sb")
                nc.scalar.activation(s0sb[0:64, :], s0lo[0:64, :], AF.Copy)
                nc.vector.tensor_copy(s0sb[64:128, :], s0hi[64:128, :])

            s0g, _ = allreduce_s(0, s0sb)
            v0 = squash(s0g, 1.0 / float(il * n_cores), DT16)

            # ------------- Routing passes (agree -> b -> e -> s-partial) --
            bdram = cdram.tile([nblk, J * g], DT32, tag="bdram")

            def routing_pass(vprev16, first):
                tag = "A" if first else "B"
                szu = keep.tile([128, F], DT32, tag="szu" + tag)
                eloc = keep.tile([1, J], DT32, tag="eloc" + tag)
                with (
                    tc.tile_pool(name="ubpool", bufs=2) as ubpool,
                    tc.tile_pool(name="ebpool", bufs=2) as ebpool,
                    tc.tile_pool(name="prpool", bufs=1) as prpool,
                    tc.tile_pool(name="smallp", bufs=1) as smallp,
                    tc.tile_pool(name="psAg", bufs=1, space="PSUM") as psAg,
                    tc.tile_pool(name="psEb", bufs=2, space="PSUM") as psEb,
                ):
                    for bk in range(nblk):
                        ublk = ubpool.tile([128, g, F], DT16)
                        nc.sync.dma_start(
                            ublk[:], u_dram[:, bk * g : (bk + 1) * g, :]
                        )
                        # agreement: b1[i,j]*B = sum_{b,v} u_hat * v_prev
                        # one K=128 matmul per (j, vl), accumulated over vl
                        agps = psAg.tile([1, J * g], DT32)
                        for j in range(J):
                            for vl in range(VL):
                                col = vl * J + j
                                nc.tensor.matmul(
                                    agps[:, j * g : (j + 1) * g],
                                    vprev16[:, col : col + 1],
                                    ublk[:, :, col],
                                    start=(vl == 0),
                                    stop=(vl == VL - 1),
                                )
                        # b_k = agree/B (+ b_prev); e = exp(b_k - SHIFT)
                        bsb = smallp.tile([1, J * g], DT32, tag="bsbB")
                        if first:
                            nc.scalar.mul(bsb[:], agps[:], 1.0 / B)
                            nc.sync.dma_start(
                                bdram[bk].rearrange("(o c) -> o c", o=1), bsb[:]
                            )
                        else:
                            b1blk = smallp.tile([1, J * g], DT32, tag="rowA")
                            nc.sync.dma_start(
                                b1blk[:], bdram[bk].rearrange("(o c) -> o c", o=1)
                            )
                            nc.vector.scalar_tensor_tensor(
                                bsb[:], agps[:], 1.0 / B, b1blk[:],
                                OP.mult, OP.add,
                            )
                        esb = smallp.tile([1, J * g], DT32, tag="rowA")
                        nc.scalar.activation(esb[:], bsb[:], AF.Exp, bias=shiftb[:])
                        # local softmax denominator accumulation
                        etmp = smallp.tile([1, J], DT32, tag="etmp")
                        nc.vector.tensor_reduce(
                            etmp[:],
                            esb[:].rearrange("p (j g) -> p j g", g=g),
                            AX.X,
                            OP.add,
                        )
                        if bk == 0:
                            nc.vector.tensor_copy(eloc[:], etmp[:])
                        else:
                            nc.vector.tensor_tensor(eloc[:], eloc[:], etmp[:], OP.add)
                        # broadcast e to all partitions (i-major) via
                        # K=1 ones-matmuls, then downcast to fp16.
                        # Two half-g chunks so the PSUM pool double-buffers.
                        # pass A: e fits fp16 (b1 in [-5,6] at full size),
                        # scaled by 2^-4 for headroom (cancels via 1/Z).
                        # pass B: b2 can reach ~33, so e spans e^44 -> bf16
                        # for e AND the products/tree (range, not precision)
                        edt = (DTB if passa_bf16 else DT16) if first else DTB
                        alpha = ALPHA_A if first else 1.0
                        eb16 = ebpool.tile([128, g, 1, J], edt, tag="eb16")
                        eview = (
                            esb[:]
                            .rearrange("p (j g) -> p j g", g=g)
                            .rearrange("p j g -> p g j")
                        )
                        gh2 = g // 2
                        gchunk = min(gh2, 512 // J)
                        for half in range(2):
                            ebps = psEb.tile([128, gh2 * J], DT32)
                            for k in range(0, gh2, gchunk):
                                kk = half * gh2 + k
                                nc.tensor.matmul(
                                    ebps[:, k * J : (k + gchunk) * J],
                                    ones_row[:],
                                    eview[:, kk : kk + gchunk, :],
                                    start=True,
                                    stop=True,
                                )
                            nc.scalar.mul(
                                eb16[:, half * gh2 : (half + 1) * gh2, :, :]
                                .rearrange("p g o j -> p (g o j)"),
                                ebps[:],
                                alpha,
                            )
                        # products into a quarter-block tile, tree-sum over i
                        gh = max(1, g // 4)
                        for hh in range(g // gh):
                            prods = prpool.tile([128, gh, F], edt, tag="prods")
                            sl = slice(hh * gh, (hh + 1) * gh)
                            nc.vector.tensor_tensor(
                                prods[:].rearrange("p g (vl j) -> p g vl j", j=J),
                                ublk[:, sl, :].rearrange(
                                    "p g (vl j) -> p g vl j", j=J
                                ),
                                eb16[:, sl, :, :].broadcast_to([128, gh, VL, J]),
                                OP.mult,
                            )
                            h = gh
                            while h > 1:
                                h //= 2
                                nc.vector.tensor_tensor(
                                    prods[:, 0:h, :],
                                    prods[:, 0:h, :],
                                    prods[:, h : 2 * h, :],
                                    OP.add,
                                )
                            if bk == 0 and hh == 0:
                                nc.vector.tensor_copy(szu[:], prods[:, 0, :])
                            else:
                                nc.vector.tensor_tensor(
                                    szu[:], szu[:], prods[:, 0, :], OP.add
                                )
                return szu, eloc

            def normalize(k, sg, zg, alpha):
                """s = s_unnorm / (alpha*Z) per j."""
                zr = keep.tile([1, J], DT32, tag=f"zr{k}")
                nc.vector.reciprocal(zr[:], zg[:])
                if alpha != 1.0:
                    nc.vector.tensor_scalar_mul(zr[:], zr[:], 1.0 / alpha)
                zb = keep.tile([128, 1, J], DT32, tag=f"zb{k}")
                with tc.tile_pool(name="psZ", bufs=1, space="PSUM") as psZ:
                    zbps = psZ.tile([128, J], DT32, tag=f"zbps{k}")
                    nc.tensor.matmul(
                        zbps[:], ones_row[:], zr[:], start=True, stop=True
                    )
                    nc.vector.tensor_copy(
                        zb[:].rearrange("p o j -> p (o j)"), zbps[:]
                    )
                s = keep.tile([128, F], DT32, tag=f"snorm{k}")
                nc.vector.tensor_tensor(
                    s[:].rearrange("p (vl j) -> p vl j", j=J),
                    sg[:].rearrange("p (vl j) -> p vl j", j=J),
                    zb[:].broadcast_to([128, VL, J]),
                    OP.mult,
                )
                return s

            s1u, e1 = routing_pass(v0, True)
            s1g, z1 = allreduce_s(1, s1u, e1)
            v1 = squash(normalize(1, s1g, z1, ALPHA_A), 1.0, DT16)

            s2u, e2 = routing_pass(v1, False)
            s2g, z2 = allreduce_s(2, s2u, e2)
            v2 = squash(normalize(2, s2g, z2, 1.0), 1.0, DT32)

            # output in device layout [vh, b, vl, j]; host transposes
            for vh in range(2):
                nc.sync.dma_start(
                    out[vh].rearrange("b vl j -> b (vl j)"),
                    v2[vh * 64 : (vh + 1) * 64, :],
                )
    return nc


def _prep_inputs(x, W, n_cores=NC):
    il = x.shape[1] // n_cores
    x16 = x.astype(np.float16)
    W16 = W.astype(np.float16)
    maps = []
    for c in range(n_cores):
        sl = slice(c * il, (c + 1) * il)
        xtc = np.ascontiguousarray(x16[:, sl, :].transpose(2, 1, 0))  # [D,il,B]
        wtc = np.ascontiguousarray(W16[sl].transpose(0, 2, 3, 1)).reshape(
            il, D, JV
        )  # [il, D, (v,j)]
        maps.append({"xt": xtc, "wt": wtc})
    return maps


def kernel(x, W):
    from concourse.bass_utils import run_bass_kernel_spmd

    if "nc" not in _CACHE:
        _CACHE["nc"] = build_kernel()
    in_maps = _prep_inputs(np.asarray(x), np.asarray(W))
    res = run_bass_kernel_spmd(_CACHE["nc"], in_maps, list(range(NC)))
    o = np.asarray(res.results[0]["out"], dtype=np.float32)  # [2, B, VL, J]
    return np.ascontiguousarray(o.transpose(1, 3, 0, 2).reshape(B, J, V))
